# revision 12
# baseline (speedup 1.0000x reference)
"""Trainium2 Bass kernel for nn_Discriminator (GRU over [64, 1024, 1024]).

Self-contained: builds an SPMD Bass/Tile kernel for 8 NeuronCores,
batch-parallel (8 batch rows per core), runs it via PJRT on the axon
devices, and applies the tiny output head on the host.

Layout ("tile-slot"): SBUF tensors are [128 partitions, (j, b)] where
hidden index hid = j*128 + p (j = k-tile 0..7), b = local batch 0..7.

v2 design: single fused loop. Per step the 192 recurrent LDW+MM pairs
run k-outer (consuming h chunks as the previous step's tail produces
them); the r/z sigmoid path splits off early inside the k=7 group and
the elementwise tail is split into two [128,32] halves so the next
step's k=0..3 matmuls start as soon as half 0 of h_new lands. The
input projection (x @ W_ih) is not a separate phase: each block
computes the NEXT block's xg (24 m-tiles at N=64) in the PE's
step-boundary dependency stalls, writing an SBUF ring - no DRAM
roundtrip, and the PE never idles long enough to re-throttle HAM.
"""

import numpy as np
import ml_dtypes

import jax
from jax.sharding import Mesh, PartitionSpec, NamedSharding
from jax.experimental.shard_map import shard_map

import concourse.bass as bass
import concourse.mybir as mybir
import concourse.tile as tile
from concourse import bacc, bass2jax
from concourse.bass import ds

F32 = mybir.dt.float32
BF16 = mybir.dt.bfloat16
FP8 = mybir.dt.float8e4
WSCALE = 32.0  # W_hh is stored fp8 e4m3 pre-scaled by 32; undone in ACT scale
AF = mybir.ActivationFunctionType
OP = mybir.AluOpType

B, T, S = 64, 1024, 1024
N_CORES = 8
BC = B // N_CORES      # 8 local batch rows
KT = S // 128          # 8 hidden k-tiles
MT = 3 * KT            # 24 gate m-tiles
JB = KT * BC           # 64 slot-layout free size
TB = 8                 # timesteps per block
NBLK = T // TB         # 128 blocks
TBBC = TB * BC         # 64 columns per block
G3S = 3 * S


def _build():
    nc = bacc.Bacc("TRN2", target_bir_lowering=False, num_devices=N_CORES)

    # xT padded by one block of zero columns so the last loop iteration's
    # lookahead x-projection reads in-bounds (its result is never used).
    xT = nc.dram_tensor("xT", [S, T * BC + TBBC], BF16, kind="ExternalInput")
    wih = nc.dram_tensor("wih", [S, 3 * S], BF16, kind="ExternalInput")
    whh = nc.dram_tensor("whh", [S, 3 * S], FP8, kind="ExternalInput")
    biasm = nc.dram_tensor("biasm", [128, MT], F32, kind="ExternalInput")
    bhhn = nc.dram_tensor("bhhn", [128, JB], F32, kind="ExternalInput")
    hT_out = nc.dram_tensor("hT", [128, JB], F32, kind="ExternalOutput")

    wih_sb = nc.alloc_sbuf_tensor("wih_sb", [128, KT * G3S], BF16)
    whh_sb = nc.alloc_sbuf_tensor("whh_sb", [128, KT * G3S], FP8)
    biasm_sb = nc.alloc_sbuf_tensor("biasm_sb", [128, MT], F32)
    bhhn_sb = nc.alloc_sbuf_tensor("bhhn_sb", [128, JB], F32)
    h_a = nc.alloc_sbuf_tensor("h_a", [128, JB], BF16)
    h_b = nc.alloc_sbuf_tensor("h_b", [128, JB], BF16)
    # rings: 2 slots each, slot = target_block % 2
    xt_ring = nc.alloc_sbuf_tensor("xt_ring", [128, 2 * KT * TBBC], BF16)
    xg_ring = nc.alloc_sbuf_tensor("xg_ring", [128, 2 * MT * TBBC], BF16)

    wihR = wih.rearrange("(k p) g -> p k g", p=128)
    whhR = whh.rearrange("(k p) g -> p k g", p=128)
    xTr = xT.rearrange("(k p) f -> p k f", p=128)
    xtv = xt_ring[:, :].rearrange("p (s k f) -> p s k f", s=2, k=KT)
    xgv = xg_ring[:, :].rearrange("p (s m f) -> p s m f", s=2, m=MT)

    with tile.TileContext(nc) as tc:
        nc.sync.dma_start(out=wih_sb[:, :].rearrange("p (k g) -> p k g", k=KT), in_=wihR)
        nc.sync.dma_start(out=whh_sb[:, :].rearrange("p (k g) -> p k g", k=KT), in_=whhR)
        nc.sync.dma_start(out=biasm_sb[:, :], in_=biasm[:, :])
        nc.sync.dma_start(out=bhhn_sb[:, :], in_=bhhn[:, :])
        nc.vector.memset(h_a[:, :], 0.0)

        with tc.tile_pool(name="sc_ps", bufs=2, space="PSUM") as sps_pool, \
             tc.tile_pool(name="p1_ps", bufs=6, space="PSUM") as p1_pool, \
             tc.tile_pool(name="sc_ew", bufs=3) as ew_pool:

            def emit_p1_mtile(m, slot):
                """One x-projection m-tile (8 LDW+MM pairs at N=64); the bias
                evac is deferred so tail DVE ops aren't stuck behind it in the
                strict-FIFO vector queue."""
                ps1 = p1_pool.tile([128, TBBC], F32, tag="p1")
                for k in range(KT):
                    nc.tensor.matmul(
                        ps1[:, :],
                        wih_sb[:, k * G3S + m * 128 : k * G3S + (m + 1) * 128],
                        xtv[:, slot, k, :],
                        start=(k == 0),
                        stop=(k == KT - 1),
                    )
                def evac():
                    # on the ACT engine (biasm pre-scaled by 32 host-side):
                    # keeps the strict-FIFO DVE queue free for the tail chain
                    nc.scalar.activation(
                        xgv[:, slot, m, :], ps1[:, :], AF.Identity,
                        bias=biasm_sb[:, m : m + 1], scale=WSCALE,
                    )
                return evac

            def emit_step(tp, slot, p1_slot, p1_ms):
                """One GRU timestep: k-outer MMs, phase-1 filler, split tail."""
                h_cur = h_a if tp % 2 == 0 else h_b
                h_nxt = h_b if tp % 2 == 0 else h_a
                # Full-bank PSUM tiles: start=True clears has_written for the
                # WHOLE bank, so with k-outer interleaved accumulation groups
                # only the first MM into each bank may carry start=True (later
                # first-writes overwrite via the cleared has_written bit).
                ps_t = sps_pool.tile([128, 512], F32, tag="ps")
                ps_rz = ps_t[:, 0:128]
                ps_n = ps_t[:, 128 : 128 + JB]

                def mm(m, k):
                    out_ap = (
                        ps_rz[:, m * BC : (m + 1) * BC]
                        if m < 16
                        else ps_n[:, (m - 16) * BC : (m - 15) * BC]
                    )
                    nc.tensor.matmul(
                        out_ap,
                        whh_sb[:, k * G3S + m * 128 : k * G3S + (m + 1) * 128],
                        h_cur[:, k * BC : (k + 1) * BC],
                        start=(k == 0 and m == 0),
                        stop=(k == KT - 1),
                    )

                # Hybrid schedule: k-outer for k=0..3 consumes h chunks as the
                # previous tail streams them in; by k=4 all chunks exist, so
                # switch m-outer to land each gate's stop early - the r/z
                # sigmoid path then overlaps the n-gate matmuls.
                for k in range(4):
                    for m in range(MT):
                        mm(m, k)
                for m in range(MT):
                    for k in range(4, KT):
                        mm(m, k)
                # phase-1 filler: PE chews these while the tail computes h_nxt
                evacs = [emit_p1_mtile(m, p1_slot) for m in p1_ms]
                # --- tail ---
                # r path first (its ps columns complete first in the k=7 group)
                rz_r = ew_pool.tile([128, 64], F32, tag="rz_r")
                nc.vector.tensor_tensor(
                    rz_r[:, :].rearrange("p (m f) -> p m f", m=8),
                    ps_rz[:, 0:64].rearrange("p (m f) -> p m f", m=8),
                    xgv[:, slot, 0:8, ds(tp * BC, BC)],
                    OP.add,
                )
                sig_r = ew_pool.tile([128, 64], BF16, tag="sig_r")
                nc.scalar.activation(sig_r[:, :], rz_r[:, :], AF.Sigmoid, scale=1.0 / WSCALE)
                rz_z = ew_pool.tile([128, 64], F32, tag="rz_z")
                nc.vector.tensor_tensor(
                    rz_z[:, :].rearrange("p (m f) -> p m f", m=8),
                    ps_rz[:, 64:128].rearrange("p (m f) -> p m f", m=8),
                    xgv[:, slot, 8:16, ds(tp * BC, BC)],
                    OP.add,
                )
                sig_z = ew_pool.tile([128, 64], BF16, tag="sig_z")
                nc.scalar.activation(sig_z[:, :], rz_z[:, :], AF.Sigmoid, scale=1.0 / WSCALE)
                for h2 in range(2):
                    c0 = h2 * 32
                    hn = ew_pool.tile([128, 32], F32, tag=f"hn{h2}")
                    nc.vector.tensor_tensor(
                        hn[:, :], ps_n[:, c0 : c0 + 32], bhhn_sb[:, c0 : c0 + 32], OP.add
                    )
                    rhn = ew_pool.tile([128, 32], F32, tag=f"rhn{h2}")
                    nc.vector.tensor_tensor(
                        rhn[:, :], hn[:, :], sig_r[:, c0 : c0 + 32], OP.mult
                    )
                    npre = ew_pool.tile([128, 32], F32, tag=f"npre{h2}")
                    nc.vector.tensor_tensor(
                        npre[:, :].rearrange("p (m f) -> p m f", m=4),
                        rhn[:, :].rearrange("p (m f) -> p m f", m=4),
                        xgv[:, slot, 16 + 4 * h2 : 20 + 4 * h2, ds(tp * BC, BC)],
                        OP.add,
                    )
                    n_t = ew_pool.tile([128, 32], BF16, tag=f"n_t{h2}")
                    nc.scalar.activation(n_t[:, :], npre[:, :], AF.Tanh, scale=1.0 / WSCALE)
                    d_t = ew_pool.tile([128, 32], BF16, tag=f"d_t{h2}")
                    nc.vector.tensor_tensor(
                        d_t[:, :], h_cur[:, c0 : c0 + 32], n_t[:, :], OP.subtract
                    )
                    zd = ew_pool.tile([128, 32], BF16, tag=f"zd{h2}")
                    nc.vector.tensor_tensor(
                        zd[:, :], d_t[:, :], sig_z[:, c0 : c0 + 32], OP.mult
                    )
                    nc.vector.tensor_tensor(
                        h_nxt[:, c0 : c0 + 32], zd[:, :], n_t[:, :], OP.add
                    )
                for ev in evacs:
                    ev()

            def emit_block(parity, xt_src_ds, p1_ms_fn):
                """One block = 8 steps. Loads xt for block b+1 (slot 1-parity...
                target slot = (b+1)%2) and computes xg(b+1) in step stalls."""
                tslot = 1 - parity  # (b+1) % 2
                nc.sync.dma_start(out=xtv[:, tslot, :, :], in_=xTr[:, :, xt_src_ds])
                for tp in range(TB):
                    emit_step(tp, parity, tslot, p1_ms_fn(tp))

            # bootstrap: xg(block 0) into ring slot 0
            nc.sync.dma_start(out=xtv[:, 0, :, :], in_=xTr[:, :, ds(0, TBBC)])
            for m in range(MT):
                emit_p1_mtile(m, 0)()

            # main loop: body = 2 blocks (A=2i even slot, B=2i+1 odd slot)
            with tc.For_i(0, NBLK // 2, 1, hint_engines=(mybir.EngineType.PE,)) as it:
                p1_sched = lambda tp: range(3 * tp, 3 * tp + 3)
                emit_block(0, ds(it * (2 * TBBC) + TBBC, TBBC), p1_sched)
                emit_block(1, ds(it * (2 * TBBC) + 2 * TBBC, TBBC), p1_sched)

        hT_sb = nc.alloc_sbuf_tensor("hT_sb", [128, JB], F32)
        nc.vector.tensor_copy(hT_sb[:, :], h_a[:, :])
        nc.sync.dma_start(out=hT_out[:, :], in_=hT_sb[:, :])

    nc.compile()
    return nc


def _prep_inputs(inputs):
    batch = np.asarray(inputs["batch"], np.float32)
    W_ih = np.asarray(inputs["W_ih"], np.float32)
    W_hh = np.asarray(inputs["W_hh"], np.float32)
    b_ih = np.asarray(inputs["b_ih"], np.float32)
    b_hh = np.asarray(inputs["b_hh"], np.float32)

    wihT = np.ascontiguousarray(W_ih.T).astype(ml_dtypes.bfloat16)
    whhT = (np.ascontiguousarray(W_hh.T) * WSCALE).astype(ml_dtypes.float8_e4m3)
    bias = b_ih.copy()
    bias[: 2 * S] += b_hh[: 2 * S]
    biasm = np.ascontiguousarray(bias.reshape(MT, 128).T * WSCALE).astype(np.float32)
    bn = b_hh[2 * S :].reshape(KT, 128).T
    bhhn = (np.repeat(bn[:, :, None], BC, axis=2).reshape(128, JB) * WSCALE).astype(np.float32)

    in_maps = []
    for c in range(N_CORES):
        bs = batch[c * BC : (c + 1) * BC]
        xTc = np.ascontiguousarray(bs.transpose(2, 1, 0).reshape(S, T * BC))
        xTc = np.concatenate(
            [xTc, np.zeros((S, TBBC), np.float32)], axis=1
        )
        in_maps.append({
            "xT": xTc.astype(ml_dtypes.bfloat16),
            "wih": wihT,
            "whh": whhT,
            "biasm": biasm,
            "bhhn": bhhn,
        })
    return in_maps


class _Runner:
    """Compile once, keep the PJRT executable; run per-core in_maps SPMD."""

    def __init__(self, nc, n_cores):
        bass2jax.install_neuronx_cc_hook()
        self.nc, self.n_cores = nc, n_cores
        pname = nc.partition_id_tensor.name if nc.partition_id_tensor else None
        in_names, out_names, out_avals = [], [], []
        for alloc in nc.m.functions[0].allocations:
            if not isinstance(alloc, mybir.MemoryLocationSet):
                continue
            name = alloc.memorylocations[0].name
            if alloc.kind == "ExternalInput":
                if name != pname:
                    in_names.append(name)
            elif alloc.kind == "ExternalOutput":
                out_names.append(name)
                out_avals.append(
                    jax.core.ShapedArray(tuple(alloc.tensor_shape), mybir.dt.np(alloc.dtype))
                )
        self.in_names, self.out_names, self.out_avals = in_names, out_names, out_avals
        n_params = len(in_names)
        all_in = list(in_names) + list(out_names)
        if pname is not None:
            all_in.append(pname)

        def _body(*args):
            operands = list(args)
            if pname is not None:
                operands.append(bass2jax.partition_id_tensor())
            return tuple(
                bass2jax._bass_exec_p.bind(
                    *operands,
                    out_avals=tuple(out_avals),
                    in_names=tuple(all_in),
                    out_names=tuple(out_names),
                    lowering_input_output_aliases=(),
                    sim_require_finite=False,
                    sim_require_nnan=False,
                    nc=nc,
                )
            )

        devices = jax.devices()[:n_cores]
        self.mesh = Mesh(np.asarray(devices), ("core",))
        specs = (PartitionSpec("core"),) * (n_params + len(out_names))
        self.fn = jax.jit(
            shard_map(_body, mesh=self.mesh, in_specs=specs,
                      out_specs=(PartitionSpec("core"),) * len(out_names),
                      check_rep=False),
            keep_unused=True,
        )

    def run(self, in_maps):
        concat = [
            np.concatenate([np.asarray(in_maps[c][n]) for c in range(self.n_cores)], axis=0)
            for n in self.in_names
        ]
        zeros = [
            np.zeros((self.n_cores * a.shape[0], *a.shape[1:]), a.dtype)
            for a in self.out_avals
        ]
        sh = NamedSharding(self.mesh, PartitionSpec("core"))
        dev = [jax.device_put(a, sh) for a in concat + zeros]
        outs = self.fn(*dev)
        jax.block_until_ready(outs)
        return [
            {
                n: np.asarray(outs[i]).reshape(self.n_cores, *self.out_avals[i].shape)[c]
                for i, n in enumerate(self.out_names)
            }
            for c in range(self.n_cores)
        ]


_CACHED = {}


def kernel(**inputs) -> np.ndarray:
    if "runner" not in _CACHED:
        _CACHED["nc"] = _build()
        _CACHED["runner"] = _Runner(_CACHED["nc"], N_CORES)
    runner = _CACHED["runner"]
    in_maps = _prep_inputs(inputs)
    results = runner.run(in_maps)

    W_out = np.asarray(inputs["W_out"], np.float32)
    b_out = np.asarray(inputs["b_out"], np.float32)
    outs = []
    for c in range(N_CORES):
        hT = np.asarray(results[c]["hT"], np.float32)
        h = hT.reshape(128, KT, BC).transpose(2, 1, 0).reshape(BC, S)
        logits = h @ W_out.T + b_out
        outs.append(1.0 / (1.0 + np.exp(-logits[:, 0])))
    return np.concatenate(outs, 0).astype(np.float32)


# revision 13
# speedup vs baseline: 1.0457x; 1.0457x over previous
"""Trainium2 Bass kernel for nn_Discriminator (GRU over [64, 1024, 1024]).

Self-contained: builds an SPMD Bass/Tile kernel for 8 NeuronCores,
batch-parallel (8 batch rows per core), runs it via PJRT on the axon
devices, and applies the tiny output head on the host.

Layout ("tile-slot"): SBUF tensors are [128 partitions, (j, b)] where
hidden index hid = j*128 + p (j = k-tile 0..7), b = local batch 0..7.

Single fused loop; no separate x-projection phase and no xg DRAM
roundtrip. Per step the 192 recurrent LDW+MM pairs run in a hybrid
order: k-outer for k=0..3 (consuming h chunks as the previous step's
tail streams them into SBUF), then m-outer for k=4..7 so each gate's
PSUM accumulation stops early and the r/z sigmoid path overlaps the
n-gate matmuls. The elementwise tail is split into two [128,32]
halves so the next step's k=0..3 matmuls start as soon as half 0 of
h_new lands. Each block computes the NEXT block's x-projection (24
m-tiles at N=64) in the PE's step-boundary dependency stalls, into an
SBUF ring; those bias-evacs run on the ACT engine (Identity w/ bias,
scale) to keep the strict-FIFO DVE queue free for the tail chain.
W_hh is fp8 e4m3 pre-scaled by 32 (undone via ACT scale at the
sigmoid/tanh) - numerically free here since the contractive GRU
dynamics wash out weight quantization, and the scan is MM-issue-bound
so fp8 mainly shrinks SBUF. PSUM: scan tiles are full-bank and only
the first MM into each bank carries start=True, because start=True
clears has_written for the WHOLE bank (interleaved accumulation
groups would otherwise lose their k=0 partials).
"""

import numpy as np
import ml_dtypes

import jax
from jax.sharding import Mesh, PartitionSpec, NamedSharding
from jax.experimental.shard_map import shard_map

import concourse.bass as bass
import concourse.mybir as mybir
import concourse.tile as tile
from concourse import bacc, bass2jax
from concourse.bass import ds

F32 = mybir.dt.float32
BF16 = mybir.dt.bfloat16
FP8 = mybir.dt.float8e4
WSCALE = 32.0  # W_hh is stored fp8 e4m3 pre-scaled by 32; undone in ACT scale
AF = mybir.ActivationFunctionType
OP = mybir.AluOpType

B, T, S = 64, 1024, 1024
N_CORES = 8
BC = B // N_CORES      # 8 local batch rows
KT = S // 128          # 8 hidden k-tiles
MT = 3 * KT            # 24 gate m-tiles
JB = KT * BC           # 64 slot-layout free size
TB = 8                 # timesteps per block
NBLK = T // TB         # 128 blocks
TBBC = TB * BC         # 64 columns per block
G3S = 3 * S


def _build():
    nc = bacc.Bacc("TRN2", target_bir_lowering=False, num_devices=N_CORES)

    # xT padded by one block of zero columns so the last loop iteration's
    # lookahead x-projection reads in-bounds (its result is never used).
    xT = nc.dram_tensor("xT", [S, T * BC + TBBC], BF16, kind="ExternalInput")
    wih = nc.dram_tensor("wih", [S, 3 * S], BF16, kind="ExternalInput")
    whh = nc.dram_tensor("whh", [S, 3 * S], FP8, kind="ExternalInput")
    biasm = nc.dram_tensor("biasm", [128, MT], F32, kind="ExternalInput")
    bhhn = nc.dram_tensor("bhhn", [128, JB], F32, kind="ExternalInput")
    hT_out = nc.dram_tensor("hT", [128, JB], F32, kind="ExternalOutput")

    wih_sb = nc.alloc_sbuf_tensor("wih_sb", [128, KT * G3S], BF16)
    whh_sb = nc.alloc_sbuf_tensor("whh_sb", [128, KT * G3S], FP8)
    biasm_sb = nc.alloc_sbuf_tensor("biasm_sb", [128, MT], F32)
    bhhn_sb = nc.alloc_sbuf_tensor("bhhn_sb", [128, JB], F32)
    h_a = nc.alloc_sbuf_tensor("h_a", [128, JB], BF16)
    h_b = nc.alloc_sbuf_tensor("h_b", [128, JB], BF16)
    # rings: 2 slots each, slot = target_block % 2
    xt_ring = nc.alloc_sbuf_tensor("xt_ring", [128, 2 * KT * TBBC], BF16)
    xg_ring = nc.alloc_sbuf_tensor("xg_ring", [128, 2 * MT * TBBC], BF16)

    wihR = wih.rearrange("(k p) g -> p k g", p=128)
    whhR = whh.rearrange("(k p) g -> p k g", p=128)
    xTr = xT.rearrange("(k p) f -> p k f", p=128)
    xtv = xt_ring[:, :].rearrange("p (s k f) -> p s k f", s=2, k=KT)
    xgv = xg_ring[:, :].rearrange("p (s m f) -> p s m f", s=2, m=MT)

    with tile.TileContext(nc) as tc:
        nc.sync.dma_start(out=wih_sb[:, :].rearrange("p (k g) -> p k g", k=KT), in_=wihR)
        nc.sync.dma_start(out=whh_sb[:, :].rearrange("p (k g) -> p k g", k=KT), in_=whhR)
        nc.sync.dma_start(out=biasm_sb[:, :], in_=biasm[:, :])
        nc.sync.dma_start(out=bhhn_sb[:, :], in_=bhhn[:, :])
        nc.vector.memset(h_a[:, :], 0.0)

        with tc.tile_pool(name="sc_ps", bufs=2, space="PSUM") as sps_pool, \
             tc.tile_pool(name="p1_ps", bufs=4, space="PSUM") as p1_pool, \
             tc.tile_pool(name="sc_ew", bufs=3) as ew_pool:

            def emit_p1_mtile(m, slot):
                """One x-projection m-tile (8 LDW+MM pairs at N=64); the bias
                evac is deferred so tail DVE ops aren't stuck behind it in the
                strict-FIFO vector queue."""
                ps1 = p1_pool.tile([128, TBBC], F32, tag="p1")
                for k in range(KT):
                    nc.tensor.matmul(
                        ps1[:, :],
                        wih_sb[:, k * G3S + m * 128 : k * G3S + (m + 1) * 128],
                        xtv[:, slot, k, :],
                        start=(k == 0),
                        stop=(k == KT - 1),
                    )
                def evac():
                    # on the ACT engine (biasm pre-scaled by 32 host-side):
                    # keeps the strict-FIFO DVE queue free for the tail chain
                    nc.scalar.activation(
                        xgv[:, slot, m, :], ps1[:, :], AF.Identity,
                        bias=biasm_sb[:, m : m + 1], scale=WSCALE,
                    )
                return evac

            def emit_step(tp, slot, p1_slot, p1_ms):
                """One GRU timestep: k-outer MMs, phase-1 filler, split tail."""
                h_cur = h_a if tp % 2 == 0 else h_b
                h_nxt = h_b if tp % 2 == 0 else h_a
                # Full-bank PSUM tiles: start=True clears has_written for the
                # WHOLE bank, so with k-outer interleaved accumulation groups
                # only the first MM into each bank may carry start=True (later
                # first-writes overwrite via the cleared has_written bit).
                ps_rz_t = sps_pool.tile([128, 512], F32, tag="ps_rz")
                ps_n_t = sps_pool.tile([128, 512], F32, tag="ps_n")
                ps_rz = ps_rz_t[:, 0:128]
                ps_n = ps_n_t[:, 0:JB]

                def mm(m, k):
                    out_ap = (
                        ps_rz[:, m * BC : (m + 1) * BC]
                        if m < 16
                        else ps_n[:, (m - 16) * BC : (m - 15) * BC]
                    )
                    nc.tensor.matmul(
                        out_ap,
                        whh_sb[:, k * G3S + m * 128 : k * G3S + (m + 1) * 128],
                        h_cur[:, k * BC : (k + 1) * BC],
                        start=(k == 0 and m in (0, 16)),
                        stop=(k == KT - 1),
                    )

                # Hybrid schedule: k-outer for k=0..3 consumes h chunks as the
                # previous tail streams them in; by k=4 all chunks exist, so
                # switch m-outer to land each gate's stop early - the r/z
                # sigmoid path then overlaps the n-gate matmuls.
                for k in range(4):
                    for m in range(MT):
                        mm(m, k)
                for m in range(MT):
                    for k in range(4, KT):
                        mm(m, k)
                # phase-1 filler: PE chews these while the tail computes h_nxt
                evacs = [emit_p1_mtile(m, p1_slot) for m in p1_ms]
                # --- tail ---
                # r path first (its ps columns complete first in the k=7 group)
                rz_r = ew_pool.tile([128, 64], F32, tag="rz_r")
                nc.vector.tensor_tensor(
                    rz_r[:, :].rearrange("p (m f) -> p m f", m=8),
                    ps_rz[:, 0:64].rearrange("p (m f) -> p m f", m=8),
                    xgv[:, slot, 0:8, ds(tp * BC, BC)],
                    OP.add,
                )
                sig_r = ew_pool.tile([128, 64], BF16, tag="sig_r")
                nc.scalar.activation(sig_r[:, :], rz_r[:, :], AF.Sigmoid, scale=1.0 / WSCALE)
                rz_z = ew_pool.tile([128, 64], F32, tag="rz_z")
                nc.vector.tensor_tensor(
                    rz_z[:, :].rearrange("p (m f) -> p m f", m=8),
                    ps_rz[:, 64:128].rearrange("p (m f) -> p m f", m=8),
                    xgv[:, slot, 8:16, ds(tp * BC, BC)],
                    OP.add,
                )
                sig_z = ew_pool.tile([128, 64], BF16, tag="sig_z")
                nc.scalar.activation(sig_z[:, :], rz_z[:, :], AF.Sigmoid, scale=1.0 / WSCALE)
                for h2 in range(2):
                    c0 = h2 * 32
                    hn = ew_pool.tile([128, 32], F32, tag=f"hn{h2}")
                    nc.vector.tensor_tensor(
                        hn[:, :], ps_n[:, c0 : c0 + 32], bhhn_sb[:, c0 : c0 + 32], OP.add
                    )
                    rhn = ew_pool.tile([128, 32], F32, tag=f"rhn{h2}")
                    nc.vector.tensor_tensor(
                        rhn[:, :], hn[:, :], sig_r[:, c0 : c0 + 32], OP.mult
                    )
                    npre = ew_pool.tile([128, 32], F32, tag=f"npre{h2}")
                    nc.vector.tensor_tensor(
                        npre[:, :].rearrange("p (m f) -> p m f", m=4),
                        rhn[:, :].rearrange("p (m f) -> p m f", m=4),
                        xgv[:, slot, 16 + 4 * h2 : 20 + 4 * h2, ds(tp * BC, BC)],
                        OP.add,
                    )
                    n_t = ew_pool.tile([128, 32], BF16, tag=f"n_t{h2}")
                    nc.scalar.activation(n_t[:, :], npre[:, :], AF.Tanh, scale=1.0 / WSCALE)
                    d_t = ew_pool.tile([128, 32], BF16, tag=f"d_t{h2}")
                    nc.vector.tensor_tensor(
                        d_t[:, :], h_cur[:, c0 : c0 + 32], n_t[:, :], OP.subtract
                    )
                    zd = ew_pool.tile([128, 32], BF16, tag=f"zd{h2}")
                    nc.vector.tensor_tensor(
                        zd[:, :], d_t[:, :], sig_z[:, c0 : c0 + 32], OP.mult
                    )
                    nc.vector.tensor_tensor(
                        h_nxt[:, c0 : c0 + 32], zd[:, :], n_t[:, :], OP.add
                    )
                for ev in evacs:
                    ev()

            def emit_block(parity, xt_src_ds, p1_ms_fn):
                """One block = 8 steps. Loads xt for block b+1 (slot 1-parity...
                target slot = (b+1)%2) and computes xg(b+1) in step stalls."""
                tslot = 1 - parity  # (b+1) % 2
                nc.sync.dma_start(out=xtv[:, tslot, :, :], in_=xTr[:, :, xt_src_ds])
                for tp in range(TB):
                    emit_step(tp, parity, tslot, p1_ms_fn(tp))

            # bootstrap: xg(block 0) into ring slot 0
            nc.sync.dma_start(out=xtv[:, 0, :, :], in_=xTr[:, :, ds(0, TBBC)])
            for m in range(MT):
                emit_p1_mtile(m, 0)()

            # main loop: body = 2 blocks (A=2i even slot, B=2i+1 odd slot)
            with tc.For_i(0, NBLK // 2, 1, hint_engines=(mybir.EngineType.PE,)) as it:
                p1_sched = lambda tp: range(3 * tp, 3 * tp + 3)
                emit_block(0, ds(it * (2 * TBBC) + TBBC, TBBC), p1_sched)
                emit_block(1, ds(it * (2 * TBBC) + 2 * TBBC, TBBC), p1_sched)

        hT_sb = nc.alloc_sbuf_tensor("hT_sb", [128, JB], F32)
        nc.vector.tensor_copy(hT_sb[:, :], h_a[:, :])
        nc.sync.dma_start(out=hT_out[:, :], in_=hT_sb[:, :])

    nc.compile()
    return nc


def _prep_inputs(inputs):
    batch = np.asarray(inputs["batch"], np.float32)
    W_ih = np.asarray(inputs["W_ih"], np.float32)
    W_hh = np.asarray(inputs["W_hh"], np.float32)
    b_ih = np.asarray(inputs["b_ih"], np.float32)
    b_hh = np.asarray(inputs["b_hh"], np.float32)

    wihT = np.ascontiguousarray(W_ih.T).astype(ml_dtypes.bfloat16)
    whhT = (np.ascontiguousarray(W_hh.T) * WSCALE).astype(ml_dtypes.float8_e4m3)
    bias = b_ih.copy()
    bias[: 2 * S] += b_hh[: 2 * S]
    biasm = np.ascontiguousarray(bias.reshape(MT, 128).T * WSCALE).astype(np.float32)
    bn = b_hh[2 * S :].reshape(KT, 128).T
    bhhn = (np.repeat(bn[:, :, None], BC, axis=2).reshape(128, JB) * WSCALE).astype(np.float32)

    in_maps = []
    for c in range(N_CORES):
        bs = batch[c * BC : (c + 1) * BC]
        xTc = np.ascontiguousarray(bs.transpose(2, 1, 0).reshape(S, T * BC))
        xTc = np.concatenate(
            [xTc, np.zeros((S, TBBC), np.float32)], axis=1
        )
        in_maps.append({
            "xT": xTc.astype(ml_dtypes.bfloat16),
            "wih": wihT,
            "whh": whhT,
            "biasm": biasm,
            "bhhn": bhhn,
        })
    return in_maps


class _Runner:
    """Compile once, keep the PJRT executable; run per-core in_maps SPMD."""

    def __init__(self, nc, n_cores):
        bass2jax.install_neuronx_cc_hook()
        self.nc, self.n_cores = nc, n_cores
        pname = nc.partition_id_tensor.name if nc.partition_id_tensor else None
        in_names, out_names, out_avals = [], [], []
        for alloc in nc.m.functions[0].allocations:
            if not isinstance(alloc, mybir.MemoryLocationSet):
                continue
            name = alloc.memorylocations[0].name
            if alloc.kind == "ExternalInput":
                if name != pname:
                    in_names.append(name)
            elif alloc.kind == "ExternalOutput":
                out_names.append(name)
                out_avals.append(
                    jax.core.ShapedArray(tuple(alloc.tensor_shape), mybir.dt.np(alloc.dtype))
                )
        self.in_names, self.out_names, self.out_avals = in_names, out_names, out_avals
        n_params = len(in_names)
        all_in = list(in_names) + list(out_names)
        if pname is not None:
            all_in.append(pname)

        def _body(*args):
            operands = list(args)
            if pname is not None:
                operands.append(bass2jax.partition_id_tensor())
            return tuple(
                bass2jax._bass_exec_p.bind(
                    *operands,
                    out_avals=tuple(out_avals),
                    in_names=tuple(all_in),
                    out_names=tuple(out_names),
                    lowering_input_output_aliases=(),
                    sim_require_finite=False,
                    sim_require_nnan=False,
                    nc=nc,
                )
            )

        devices = jax.devices()[:n_cores]
        self.mesh = Mesh(np.asarray(devices), ("core",))
        specs = (PartitionSpec("core"),) * (n_params + len(out_names))
        self.fn = jax.jit(
            shard_map(_body, mesh=self.mesh, in_specs=specs,
                      out_specs=(PartitionSpec("core"),) * len(out_names),
                      check_rep=False),
            keep_unused=True,
        )

    def run(self, in_maps):
        concat = [
            np.concatenate([np.asarray(in_maps[c][n]) for c in range(self.n_cores)], axis=0)
            for n in self.in_names
        ]
        zeros = [
            np.zeros((self.n_cores * a.shape[0], *a.shape[1:]), a.dtype)
            for a in self.out_avals
        ]
        sh = NamedSharding(self.mesh, PartitionSpec("core"))
        dev = [jax.device_put(a, sh) for a in concat + zeros]
        outs = self.fn(*dev)
        jax.block_until_ready(outs)
        return [
            {
                n: np.asarray(outs[i]).reshape(self.n_cores, *self.out_avals[i].shape)[c]
                for i, n in enumerate(self.out_names)
            }
            for c in range(self.n_cores)
        ]


_CACHED = {}


def kernel(**inputs) -> np.ndarray:
    if "runner" not in _CACHED:
        _CACHED["nc"] = _build()
        _CACHED["runner"] = _Runner(_CACHED["nc"], N_CORES)
    runner = _CACHED["runner"]
    in_maps = _prep_inputs(inputs)
    results = runner.run(in_maps)

    W_out = np.asarray(inputs["W_out"], np.float32)
    b_out = np.asarray(inputs["b_out"], np.float32)
    outs = []
    for c in range(N_CORES):
        hT = np.asarray(results[c]["hT"], np.float32)
        h = hT.reshape(128, KT, BC).transpose(2, 1, 0).reshape(BC, S)
        logits = h @ W_out.T + b_out
        outs.append(1.0 / (1.0 + np.exp(-logits[:, 0])))
    return np.concatenate(outs, 0).astype(np.float32)
